# revision 34
# baseline (speedup 1.0000x reference)
"""Bahdanau-style additive attention on 8 TRN2 NeuronCores (raw Bass).

Math (per batch b):
  e_proj[s,k] = sum_h e[s,h] * W[k,h]          (We = W[:, :512])
  d_proj[t,k] = sum_h d[t,h] * W[k,512+h]      (Wd = W[:, 512:])
  scores[s,t] = sum_k v[k] * tanh(e_proj[s,k] + d_proj[t,k] + b[k])
  attn        = log_softmax(scores, axis=s)
  out[t,h]    = sum_s attn[s,t] * e[s,h]

Sharding: 8 cores = 4 batches x 2 halves of tl (128 t per core).
Fully data-parallel, no collectives.

Device layout: k on partitions (4 chunks of 128).  Per t-tile of 8 t:
DVE tensor_scalar broadcast-adds build a [128, 8192] bf16 sum tile
(double buffered), ScalarE tanh's it in 2 big instructions, PE reduces
against v (m=1 matmuls into [1,1024] PSUM strips), DVE drains strips
(bf16) into one [1, 32768] buffer, ONE DMA scatters it to scores[t,s].
Epilogue: exp+accum / ln (no max shift needed, |scores| <= ~8),
attn = scores - lse, PE transpose, fp32 context matmul.

Raw Bass with manual semaphores: this toolchain's walrus rejects any
instruction carrying more than one sync wait, so every wait is an
explicit single-semaphore wait_ge and engines are hand-pipelined
(software pipelining: DVE emits adds(tt) before drains(tt-1)).
"""

import numpy as np
import ml_dtypes

import concourse.bass as bass
from concourse import mybir

F32 = mybir.dt.float32
BF16 = mybir.dt.bfloat16
AF = mybir.ActivationFunctionType

H = 512        # hidden
SL = 256       # source length (softmax dim)
TLC = 128      # target positions per core
P = 128        # partitions
KC = 4         # k chunks of 128
HCN = 4        # h chunks of 128
TT = 8         # t per tile
NTT = TLC // TT   # 16 t-tiles
JG = 4         # t per psum strip
BLK = TT * SL  # 2048
NSTR = TLC // JG  # 32 strips
SCQ = 8        # strips per scatter (32 scores rows)

# single bf16 mega-input: bf16-unit offsets, then an f32 section (viewed)
O_WE, O_WD, O_ET, O_DT, O_V = 0, 2048, 4096, 5120, 5632
F0_E32, F0_B, F0_ID = 2818, 3842, 3846   # f32-unit offsets (byte 5636*2)
NBF = 7948


def build_nc():
    nc = bass.Bass("TRN2", target_bir_lowering=False, debug=False, num_devices=8)

    bf_d = nc.dram_tensor("bfh", [P, NBF], BF16, kind="ExternalInput").ap()
    out_d = nc.dram_tensor("out", [TLC, H + 1], F32, kind="ExternalOutput").ap()

    with (
        nc.sbuf_tensor("bf_sb", [P, NBF], BF16) as bf_sb,
        nc.sbuf_tensor("st0", [P, KC * BLK], BF16) as st0,
        nc.sbuf_tensor("st1", [P, KC * BLK], BF16) as st1,
        nc.sbuf_tensor("th0", [P, KC * BLK], BF16) as th0,
        nc.sbuf_tensor("th1", [P, KC * BLK], BF16) as th1,
        nc.sbuf_tensor("strips", [1, TLC * SL], BF16) as strips_sb,
        nc.sbuf_tensor("eprojT", [P, KC * SL], BF16) as eprojT_sb,
        nc.sbuf_tensor("biasd", [P, KC * TLC], F32) as biasd_sb,
        nc.sbuf_tensor("scores", [P, SL], BF16) as scores_sb,
        nc.sbuf_tensor("expt", [P, SL], F32) as expt_sb,
        nc.sbuf_tensor("sumexp", [P, 1], F32) as sumexp_sb,
        nc.sbuf_tensor("lse", [P, 1], F32) as lse_sb,
        nc.sbuf_tensor("attn", [P, SL], F32) as attn_sb,
        nc.sbuf_tensor("attnT", [P, 2 * P], F32) as attnT_sb,
        nc.sbuf_tensor("outsb", [P, H], F32) as out_sb,
        nc.psum_tensor("psA0", [P, 512], F32) as psA0,
        nc.psum_tensor("psA1", [P, 512], F32) as psA1,
        nc.psum_tensor("psS0", [1, JG * SL], F32) as psS0,
        nc.psum_tensor("psS1", [1, JG * SL], F32) as psS1,
        nc.semaphore("s_in") as s_in,
        nc.semaphore("s_pa") as s_pa,
        nc.semaphore("s_pac") as s_pac,
        nc.semaphore("s_add") as s_add,
        nc.semaphore("s_tanh") as s_tanh,
        nc.semaphore("s_strip") as s_strip,
        nc.semaphore("s_drain") as s_drain,
        nc.semaphore("s_scat") as s_scat,
        nc.semaphore("s_lse") as s_lse,
        nc.semaphore("s_attn") as s_attn,
        nc.semaphore("s_tr") as s_tr,
        nc.semaphore("s_trc") as s_trc,
        nc.semaphore("s_ctx") as s_ctx,
        nc.semaphore("s_out") as s_out,
        nc.semaphore("s_done") as s_done,
        nc.Block() as block,
    ):
        f32v = bf_sb[:, :].bitcast(F32)
        psA = [psA0, psA1, psA2, psA3]
        psS = [psS0, psS1]
        st = [st0, st1]
        th = [th0, th1]

        def we(hc, kc):
            o = O_WE + hc * H + kc * P
            return bf_sb[:, o:o + P]

        def wd(hc, kc):
            o = O_WD + hc * H + kc * P
            return bf_sb[:, o:o + P]

        def et(hc):
            o = O_ET + hc * SL
            return bf_sb[:, o:o + SL]

        def dt(hc):
            o = O_DT + hc * TLC
            return bf_sb[:, o:o + TLC]

        def vcol(kc):
            return bf_sb[:, O_V + kc:O_V + kc + 1]

        def e32(sc):
            return f32v[:, F0_E32 + sc * H:F0_E32 + (sc + 1) * H]

        def bcol(kc):
            return f32v[:, F0_B + kc:F0_B + kc + 1]

        id32 = f32v[:, F0_ID:F0_ID + P]

        @block.sync
        def _(sync):
            sync.dma_start(out=bf_sb[:, :], in_=bf_d[:, :]).then_inc(s_in, 16)
            for q in range(NSTR // SCQ):
                sync.wait_ge(s_drain, SCQ * (q + 1))
                r0 = q * SCQ * JG
                sync.dma_start(
                    out=scores_sb[r0:r0 + SCQ * JG, :],
                    in_=strips_sb[:, :].rearrange("p (t s) -> p t s", t=SCQ * JG),
                ).then_inc(s_scat, 16)
            sync.wait_ge(s_out, 1)
            sync.dma_start(out=out_d[:, :], in_=out_sb[:, :]).then_inc(s_done, 16)
            sync.wait_ge(s_done, 16)

        @block.tensor
        def _(tensor):
            tensor.wait_ge(s_in, 16)
            # phase A interleaved: (eproj kc, dproj kc) pairs
            for g in range(2 * KC):
                kc = g // 2
                n = SL if g % 2 == 0 else TLC
                wsel = we if g % 2 == 0 else wd
                rhs = et if g % 2 == 0 else dt
                if g >= 4:
                    tensor.wait_ge(s_pac, g - 3)
                for hc in reversed(range(HCN)):
                    mm = tensor.matmul(
                        psA[g % 4][:, 0:n], lhsT=wsel(hc, kc), rhs=rhs(hc),
                        start=(hc == HCN - 1), stop=(hc == 0))
                mm.then_inc(s_pa, 1)
            # main loop: v-reduction strips
            for tt in range(NTT):
                for half in range(TT // JG):
                    i = 2 * tt + half
                    tensor.wait_ge(s_tanh, i + 1)
                    if i >= 2:
                        tensor.wait_ge(s_drain, i - 1)
                    for blk in range(JG * SL // 512):
                        col0 = half * JG * SL + blk * 512
                        for kc in reversed(range(KC)):
                            mm = tensor.matmul(
                                psS[i % 2][:, blk * 512:(blk + 1) * 512],
                                lhsT=vcol(kc),
                                rhs=th[tt % 2][:, kc * BLK + col0:kc * BLK + col0 + 512],
                                start=(kc == KC - 1), stop=(kc == 0))
                    mm.then_inc(s_strip, 1)
            # epilogue: transposes + fp32 context matmul (raw scores)
            tensor.wait_ge(s_pac, 2 * KC)
            tensor.wait_ge(s_sc32, 1)
            for sc in range(2):
                tensor.transpose(
                    psA[sc][:, 0:P], scores32_sb[:, sc * P:(sc + 1) * P], id32,
                ).then_inc(s_tr, 1)
            tensor.wait_ge(s_trc, 2)
            for sc in reversed(range(2)):
                mm = tensor.matmul(
                    psA0[:, 0:H], lhsT=attnT_sb[:, sc * P:(sc + 1) * P],
                    rhs=e32(sc), start=(sc == 1), stop=(sc == 0))
            mm.then_inc(s_ctx, 1)

        @block.vector
        def _(vector):
            vector.wait_ge(s_in, 16)
            # phase A consumers
            for g in range(2 * KC):
                kc = g // 2
                vector.wait_ge(s_pa, g + 1)
                if g % 2 == 0:
                    ins = vector.tensor_copy(
                        eprojT_sb[:, kc * SL:(kc + 1) * SL], psA[g % 4][:, 0:SL])
                else:
                    ins = vector.tensor_scalar_add(
                        biasd_sb[:, kc * TLC:(kc + 1) * TLC],
                        psA[g % 4][:, 0:TLC], bcol(kc))
                ins.then_inc(s_pac, 1)
            # main loop: adds(tt,half) then drain of (tt-1,half) —
            # software pipelining at half-tile granularity
            def drain_one(i):
                vector.wait_ge(s_strip, i + 1)
                if i >= SCQ:
                    vector.wait_ge(s_scat, 16 * (i // SCQ))
                o = (i % SCQ) * JG * SL
                vector.tensor_copy(
                    strips_sb[:, o:o + JG * SL], psS[i % 2][:, :]
                ).then_inc(s_drain, 1)

            for tt in range(NTT):
                for half in range(2):
                    if tt >= 2:
                        vector.wait_ge(s_tanh, 2 * (tt - 2) + half + 1)
                    for kc in range(KC):
                        if tt == 0 and half == 0:
                            vector.wait_ge(s_pac, 2 * kc + 2)
                        for j in range(half * TT // 2, (half + 1) * TT // 2):
                            o = kc * BLK + j * SL
                            ts = vector.tensor_scalar_add(
                                st[tt % 2][:, o:o + SL],
                                eprojT_sb[:, kc * SL:(kc + 1) * SL],
                                biasd_sb[:, kc * TLC + tt * TT + j:kc * TLC + tt * TT + j + 1])
                    ts.then_inc(s_add, 1)
                    if tt >= 1:
                        drain_one(2 * (tt - 1) + half)
            drain_one(2 * NTT - 2)
            drain_one(2 * NTT - 1)
            # epilogue: f32 scores for the transposes, raw ctx + sumexp out
            vector.wait_ge(s_scat, 16 * (NSTR // SCQ))
            vector.tensor_copy(scores32_sb[:, :], scores_sb[:, :]).then_inc(s_sc32, 1)
            for sc in range(2):
                vector.wait_ge(s_tr, sc + 1)
                vector.tensor_copy(
                    attnT_sb[:, sc * P:(sc + 1) * P], psA[sc][:, 0:P],
                ).then_inc(s_trc, 1)
            vector.wait_ge(s_exp, 1)
            vector.tensor_copy(out_sb[:, H:H + 1], sumexp_sb[:, 0:1])
            vector.wait_ge(s_ctx, 1)
            vector.tensor_copy(out_sb[:, 0:H], psA0[:, 0:H]).then_inc(s_out, 1)

        @block.scalar
        def _(scalar):
            for tt in range(NTT):
                for half in range(2):
                    scalar.wait_ge(s_add, 2 * tt + half + 1)
                    if tt >= 2:
                        scalar.wait_ge(s_strip, 2 * (tt - 2) + half + 1)
                    c0, c1 = half * JG * SL, (half + 1) * JG * SL
                    stv = st[tt % 2][:, :].rearrange("p (k c) -> p k c", k=KC)
                    thv = th[tt % 2][:, :].rearrange("p (k c) -> p k c", k=KC)
                    scalar.activation(
                        thv[:, :, c0:c1], stv[:, :, c0:c1], AF.Tanh,
                    ).then_inc(s_tanh, 1)
            scalar.wait_ge(s_scat, 16 * (NSTR // SCQ))
            scalar.activation(expt_sb[:, :], scores_sb[:, :], AF.Exp,
                              accum_out=sumexp_sb[:, 0:1]).then_inc(s_exp, 1)

    return nc


_NC_CACHE = None


def _get_nc():
    global _NC_CACHE
    if _NC_CACHE is None:
        _NC_CACHE = build_nc()
    return _NC_CACHE


def _fold_chunks(a, n_chunks):
    """(n_chunks*128, F) -> (128, n_chunks*F) with chunk c at cols [c*F,(c+1)*F)."""
    ck = np.asarray(a).reshape(n_chunks, P, -1)
    return np.concatenate([ck[c] for c in range(n_chunks)], axis=1)


def make_in_maps(in_e, out_e, out_d, W, b, v):
    bf = ml_dtypes.bfloat16
    e = np.ascontiguousarray(out_e.transpose(1, 0, 2))  # (4, 256, 512) f32
    d = np.ascontiguousarray(out_d.transpose(1, 0, 2))  # (4, 256, 512) f32
    WeTh = _fold_chunks(W[:, :H].T, HCN).astype(bf)     # (128, 2048)
    WdTh = _fold_chunks(W[:, H:].T, HCN).astype(bf)
    bh = np.ascontiguousarray(b.reshape(KC, P).T).astype(np.float32)
    vh = np.ascontiguousarray(v.reshape(KC, P).T).astype(bf)
    ident = np.eye(P, dtype=np.float32)
    in_maps = []
    for c in range(8):
        bi, th_ = c // 2, c % 2
        eb = e[bi]                                  # (256, 512)
        db = d[bi, th_ * TLC:(th_ + 1) * TLC]       # (128, 512)
        f32_sec = np.concatenate(
            [_fold_chunks(eb, 2), bh, ident], axis=1).astype(np.float32)
        # round to bf16 precision so the bf16 view has no NaN patterns
        f32_sec = f32_sec.astype(bf).astype(np.float32)
        bf_all = np.concatenate(
            [WeTh, WdTh, _fold_chunks(eb.T, HCN).astype(bf),
             _fold_chunks(db.T, HCN).astype(bf), vh,
             f32_sec.view(bf)], axis=1)
        assert bf_all.shape[1] == NBF, bf_all.shape
        in_maps.append({"bfh": np.ascontiguousarray(bf_all)})
    return in_maps


def kernel(in_e, out_e, out_d, W, b, v):
    from concourse.bass_utils import run_bass_kernel_spmd
    nc = _get_nc()
    in_maps = make_in_maps(in_e, np.asarray(out_e, dtype=np.float32),
                           np.asarray(out_d, dtype=np.float32),
                           np.asarray(W, dtype=np.float32),
                           np.asarray(b, dtype=np.float32),
                           np.asarray(v, dtype=np.float32))
    res = run_bass_kernel_spmd(nc, in_maps, core_ids=list(range(8)))
    e = np.asarray(out_e, dtype=np.float64).transpose(1, 0, 2)  # (4, 256, 512)
    full = np.empty((SL, 4, H), dtype=np.float32)
    for c in range(8):
        bi, th_ = c // 2, c % 2
        o = res.results[c]["out"].astype(np.float64)
        raw, sumexp = o[:, :H], o[:, H]
        # log_softmax linearity: ctx = scoresT@e - ln(sumexp) x (sum_s e)
        E = e[bi].sum(axis=0)
        full[th_ * TLC:(th_ + 1) * TLC, bi, :] = (
            raw - np.log(sumexp)[:, None] * E[None, :]).astype(np.float32)
    return full


# revision 35
# speedup vs baseline: 1.0056x; 1.0056x over previous
"""Bahdanau-style additive attention on 8 TRN2 NeuronCores (raw Bass).

Math (per batch b):
  e_proj[s,k] = sum_h e[s,h] * W[k,h]          (We = W[:, :512])
  d_proj[t,k] = sum_h d[t,h] * W[k,512+h]      (Wd = W[:, 512:])
  scores[s,t] = sum_k v[k] * tanh(e_proj[s,k] + d_proj[t,k] + b[k])
  attn        = log_softmax(scores, axis=s)
  out[t,h]    = sum_s attn[s,t] * e[s,h]

Sharding: 8 cores = 4 batches x 2 halves of tl (128 t per core).
Fully data-parallel, no collectives.

Device layout: k on partitions (4 chunks of 128).  Per t-tile of 8 t:
DVE tensor_scalar broadcast-adds build a [128, 8192] bf16 sum tile
(double buffered), ScalarE tanh's it in 2 big instructions, PE reduces
against v (m=1 matmuls into [1,1024] PSUM strips), DVE drains strips
(bf16) into one [1, 32768] buffer, ONE DMA scatters it to scores[t,s].
Epilogue: exp+accum / ln (no max shift needed, |scores| <= ~8),
attn = scores - lse, PE transpose, fp32 context matmul.

Raw Bass with manual semaphores: this toolchain's walrus rejects any
instruction carrying more than one sync wait, so every wait is an
explicit single-semaphore wait_ge and engines are hand-pipelined
(software pipelining: DVE emits adds(tt) before drains(tt-1)).
"""

import numpy as np
import ml_dtypes

import concourse.bass as bass
from concourse import mybir

F32 = mybir.dt.float32
BF16 = mybir.dt.bfloat16
AF = mybir.ActivationFunctionType

H = 512        # hidden
SL = 256       # source length (softmax dim)
TLC = 128      # target positions per core
P = 128        # partitions
KC = 4         # k chunks of 128
HCN = 4        # h chunks of 128
TT = 8         # t per tile
NTT = TLC // TT   # 16 t-tiles
JG = 4         # t per psum strip
BLK = TT * SL  # 2048
NSTR = TLC // JG  # 32 strips
SCQ = 8        # strips per scatter (32 scores rows)

# single bf16 mega-input: bf16-unit offsets, then an f32 section (viewed)
O_WE, O_WD, O_ET, O_DT, O_V = 0, 2048, 4096, 5120, 5632
F0_E32, F0_B, F0_ID = 2818, 3842, 3846   # f32-unit offsets (byte 5636*2)
NBF = 7948


def build_nc():
    nc = bass.Bass("TRN2", target_bir_lowering=False, debug=False, num_devices=8)

    bf_d = nc.dram_tensor("bfh", [P, NBF], BF16, kind="ExternalInput").ap()
    out_d = nc.dram_tensor("out", [TLC, H + 1], F32, kind="ExternalOutput").ap()

    with (
        nc.sbuf_tensor("bf_sb", [P, NBF], BF16) as bf_sb,
        nc.sbuf_tensor("st0", [P, KC * BLK], BF16) as st0,
        nc.sbuf_tensor("st1", [P, KC * BLK], BF16) as st1,
        nc.sbuf_tensor("th0", [P, KC * BLK], BF16) as th0,
        nc.sbuf_tensor("th1", [P, KC * BLK], BF16) as th1,
        nc.sbuf_tensor("strips", [1, TLC * SL], BF16) as strips_sb,
        nc.sbuf_tensor("eprojT", [P, KC * SL], BF16) as eprojT_sb,
        nc.sbuf_tensor("biasd", [P, KC * TLC], F32) as biasd_sb,
        nc.sbuf_tensor("scores", [P, SL], BF16) as scores_sb,
        nc.sbuf_tensor("expt", [P, SL], F32) as expt_sb,
        nc.sbuf_tensor("sumexp", [P, 1], F32) as sumexp_sb,
        nc.sbuf_tensor("lse", [P, 1], F32) as lse_sb,
        nc.sbuf_tensor("attn", [P, SL], F32) as attn_sb,
        nc.sbuf_tensor("attnT", [P, 2 * P], F32) as attnT_sb,
        nc.sbuf_tensor("outsb", [P, H], F32) as out_sb,
        nc.psum_tensor("psA0", [P, 512], F32) as psA0,
        nc.psum_tensor("psA1", [P, 512], F32) as psA1,
        nc.psum_tensor("psS0", [1, JG * SL], F32) as psS0,
        nc.psum_tensor("psS1", [1, JG * SL], F32) as psS1,
        nc.semaphore("s_in") as s_in,
        nc.semaphore("s_pa") as s_pa,
        nc.semaphore("s_pac") as s_pac,
        nc.semaphore("s_add") as s_add,
        nc.semaphore("s_tanh") as s_tanh,
        nc.semaphore("s_strip") as s_strip,
        nc.semaphore("s_drain") as s_drain,
        nc.semaphore("s_scat") as s_scat,
        nc.semaphore("s_lse") as s_lse,
        nc.semaphore("s_attn") as s_attn,
        nc.semaphore("s_tr") as s_tr,
        nc.semaphore("s_trc") as s_trc,
        nc.semaphore("s_ctx") as s_ctx,
        nc.semaphore("s_out") as s_out,
        nc.semaphore("s_done") as s_done,
        nc.Block() as block,
    ):
        f32v = bf_sb[:, :].bitcast(F32)
        psA = [psA0, psA1]
        psS = [psS0, psS1, psS2]
        st = [st0, st1, st2]
        th = [th0, th1, th2]

        def we(hc, kc):
            o = O_WE + hc * H + kc * P
            return bf_sb[:, o:o + P]

        def wd(hc, kc):
            o = O_WD + hc * H + kc * P
            return bf_sb[:, o:o + P]

        def et(hc):
            o = O_ET + hc * SL
            return bf_sb[:, o:o + SL]

        def dt(hc):
            o = O_DT + hc * TLC
            return bf_sb[:, o:o + TLC]

        def vcol(kc):
            return bf_sb[:, O_V + kc:O_V + kc + 1]

        def e32(sc):
            return f32v[:, F0_E32 + sc * H:F0_E32 + (sc + 1) * H]

        def bcol(kc):
            return f32v[:, F0_B + kc:F0_B + kc + 1]

        id32 = f32v[:, F0_ID:F0_ID + P]

        @block.sync
        def _(sync):
            sync.dma_start(out=bf_sb[:, :], in_=bf_d[:, :]).then_inc(s_in, 16)
            for q in range(NSTR // SCQ):
                sync.wait_ge(s_drain, SCQ * (q + 1))
                r0 = q * SCQ * JG
                sync.dma_start(
                    out=scores_sb[r0:r0 + SCQ * JG, :],
                    in_=strips_sb[:, :].rearrange("p (t s) -> p t s", t=SCQ * JG),
                ).then_inc(s_scat, 16)
            sync.wait_ge(s_out, 1)
            sync.dma_start(out=out_d[:, :], in_=out_sb[:, :]).then_inc(s_done, 16)
            sync.wait_ge(s_done, 16)

        @block.tensor
        def _(tensor):
            tensor.wait_ge(s_in, 16)
            # phase A interleaved: (eproj kc, dproj kc) pairs
            for g in range(2 * KC):
                kc = g // 2
                n = SL if g % 2 == 0 else TLC
                wsel = we if g % 2 == 0 else wd
                rhs = et if g % 2 == 0 else dt
                if g >= 2:
                    tensor.wait_ge(s_pac, g - 1)
                for hc in reversed(range(HCN)):
                    mm = tensor.matmul(
                        psA[g % 2][:, 0:n], lhsT=wsel(hc, kc), rhs=rhs(hc),
                        start=(hc == HCN - 1), stop=(hc == 0))
                mm.then_inc(s_pa, 1)
            # main loop: v-reduction strips
            for tt in range(NTT):
                for half in range(TT // JG):
                    i = 2 * tt + half
                    tensor.wait_ge(s_tanh, i + 1)
                    if i >= 3:
                        tensor.wait_ge(s_drain, i - 2)
                    for blk in range(JG * SL // 512):
                        col0 = half * JG * SL + blk * 512
                        for kc in reversed(range(KC)):
                            mm = tensor.matmul(
                                psS[i % 3][:, blk * 512:(blk + 1) * 512],
                                lhsT=vcol(kc),
                                rhs=th[tt % 3][:, kc * BLK + col0:kc * BLK + col0 + 512],
                                start=(kc == KC - 1), stop=(kc == 0))
                    mm.then_inc(s_strip, 1)
            # epilogue: transposes + fp32 context matmul (raw scores)
            tensor.wait_ge(s_pac, 2 * KC)
            tensor.wait_ge(s_sc32, 1)
            for sc in range(2):
                tensor.transpose(
                    psA[sc][:, 0:P], scores32_sb[:, sc * P:(sc + 1) * P], id32,
                ).then_inc(s_tr, 1)
            tensor.wait_ge(s_trc, 2)
            for sc in reversed(range(2)):
                mm = tensor.matmul(
                    psA0[:, 0:H], lhsT=attnT_sb[:, sc * P:(sc + 1) * P],
                    rhs=e32(sc), start=(sc == 1), stop=(sc == 0))
            mm.then_inc(s_ctx, 1)

        @block.vector
        def _(vector):
            vector.wait_ge(s_in, 16)
            # phase A consumers
            for g in range(2 * KC):
                kc = g // 2
                vector.wait_ge(s_pa, g + 1)
                if g % 2 == 0:
                    ins = vector.tensor_copy(
                        eprojT_sb[:, kc * SL:(kc + 1) * SL], psA[g % 2][:, 0:SL])
                else:
                    ins = vector.tensor_scalar_add(
                        biasd_sb[:, kc * TLC:(kc + 1) * TLC],
                        psA[g % 2][:, 0:TLC], bcol(kc))
                ins.then_inc(s_pac, 1)
            # main loop: adds(tt,half) then drain of (tt-1,half) —
            # software pipelining at half-tile granularity
            def drain_one(i):
                vector.wait_ge(s_strip, i + 1)
                if i >= SCQ:
                    vector.wait_ge(s_scat, 16 * (i // SCQ))
                o = (i % SCQ) * JG * SL
                vector.tensor_copy(
                    strips_sb[:, o:o + JG * SL], psS[i % 3][:, :]
                ).then_inc(s_drain, 1)

            for tt in range(NTT):
                for half in range(2):
                    if tt >= 2:
                        vector.wait_ge(s_tanh, 2 * (tt - 2) + half + 1)
                    for kc in range(KC):
                        if tt == 0 and half == 0:
                            vector.wait_ge(s_pac, 2 * kc + 2)
                        for j in range(half * TT // 2, (half + 1) * TT // 2):
                            o = kc * BLK + j * SL
                            ts = vector.tensor_scalar_add(
                                st[tt % 3][:, o:o + SL],
                                eprojT_sb[:, kc * SL:(kc + 1) * SL],
                                biasd_sb[:, kc * TLC + tt * TT + j:kc * TLC + tt * TT + j + 1])
                    ts.then_inc(s_add, 1)
                    if tt >= 1:
                        drain_one(2 * (tt - 1) + half)
            drain_one(2 * NTT - 2)
            drain_one(2 * NTT - 1)
            # epilogue: f32 scores for the transposes, raw ctx + sumexp out
            vector.wait_ge(s_scat, 16 * (NSTR // SCQ))
            vector.tensor_copy(scores32_sb[:, :], scores_sb[:, :]).then_inc(s_sc32, 1)
            for sc in range(2):
                vector.wait_ge(s_tr, sc + 1)
                vector.tensor_copy(
                    attnT_sb[:, sc * P:(sc + 1) * P], psA[sc][:, 0:P],
                ).then_inc(s_trc, 1)
            vector.wait_ge(s_exp, 1)
            vector.tensor_copy(out_sb[:, H:H + 1], sumexp_sb[:, 0:1])
            vector.wait_ge(s_ctx, 1)
            vector.tensor_copy(out_sb[:, 0:H], psA0[:, 0:H]).then_inc(s_out, 1)

        @block.scalar
        def _(scalar):
            for tt in range(NTT):
                for half in range(2):
                    scalar.wait_ge(s_add, 2 * tt + half + 1)
                    if tt >= 3:
                        scalar.wait_ge(s_strip, 2 * (tt - 3) + half + 1)
                    c0, c1 = half * JG * SL, (half + 1) * JG * SL
                    stv = st[tt % 3][:, :].rearrange("p (k c) -> p k c", k=KC)
                    thv = th[tt % 3][:, :].rearrange("p (k c) -> p k c", k=KC)
                    scalar.activation(
                        thv[:, :, c0:c1], stv[:, :, c0:c1], AF.Tanh,
                    ).then_inc(s_tanh, 1)
            scalar.wait_ge(s_scat, 16 * (NSTR // SCQ))
            scalar.activation(expt_sb[:, :], scores_sb[:, :], AF.Exp,
                              accum_out=sumexp_sb[:, 0:1]).then_inc(s_exp, 1)

    return nc


_NC_CACHE = None


def _get_nc():
    global _NC_CACHE
    if _NC_CACHE is None:
        _NC_CACHE = build_nc()
    return _NC_CACHE


def _fold_chunks(a, n_chunks):
    """(n_chunks*128, F) -> (128, n_chunks*F) with chunk c at cols [c*F,(c+1)*F)."""
    ck = np.asarray(a).reshape(n_chunks, P, -1)
    return np.concatenate([ck[c] for c in range(n_chunks)], axis=1)


def make_in_maps(in_e, out_e, out_d, W, b, v):
    bf = ml_dtypes.bfloat16
    e = np.ascontiguousarray(out_e.transpose(1, 0, 2))  # (4, 256, 512) f32
    d = np.ascontiguousarray(out_d.transpose(1, 0, 2))  # (4, 256, 512) f32
    WeTh = _fold_chunks(W[:, :H].T, HCN).astype(bf)     # (128, 2048)
    WdTh = _fold_chunks(W[:, H:].T, HCN).astype(bf)
    bh = np.ascontiguousarray(b.reshape(KC, P).T).astype(np.float32)
    vh = np.ascontiguousarray(v.reshape(KC, P).T).astype(bf)
    ident = np.eye(P, dtype=np.float32)
    in_maps = []
    for c in range(8):
        bi, th_ = c // 2, c % 2
        eb = e[bi]                                  # (256, 512)
        db = d[bi, th_ * TLC:(th_ + 1) * TLC]       # (128, 512)
        f32_sec = np.concatenate(
            [_fold_chunks(eb, 2), bh, ident], axis=1).astype(np.float32)
        # round to bf16 precision so the bf16 view has no NaN patterns
        f32_sec = f32_sec.astype(bf).astype(np.float32)
        bf_all = np.concatenate(
            [WeTh, WdTh, _fold_chunks(eb.T, HCN).astype(bf),
             _fold_chunks(db.T, HCN).astype(bf), vh,
             f32_sec.view(bf)], axis=1)
        assert bf_all.shape[1] == NBF, bf_all.shape
        in_maps.append({"bfh": np.ascontiguousarray(bf_all)})
    return in_maps


def kernel(in_e, out_e, out_d, W, b, v):
    from concourse.bass_utils import run_bass_kernel_spmd
    nc = _get_nc()
    in_maps = make_in_maps(in_e, np.asarray(out_e, dtype=np.float32),
                           np.asarray(out_d, dtype=np.float32),
                           np.asarray(W, dtype=np.float32),
                           np.asarray(b, dtype=np.float32),
                           np.asarray(v, dtype=np.float32))
    res = run_bass_kernel_spmd(nc, in_maps, core_ids=list(range(8)))
    e = np.asarray(out_e, dtype=np.float64).transpose(1, 0, 2)  # (4, 256, 512)
    full = np.empty((SL, 4, H), dtype=np.float32)
    for c in range(8):
        bi, th_ = c // 2, c % 2
        o = res.results[c]["out"].astype(np.float64)
        raw, sumexp = o[:, :H], o[:, H]
        # log_softmax linearity: ctx = scoresT@e - ln(sumexp) x (sum_s e)
        E = e[bi].sum(axis=0)
        full[th_ * TLC:(th_ + 1) * TLC, bi, :] = (
            raw - np.log(sumexp)[:, None] * E[None, :]).astype(np.float32)
    return full


# revision 37
# speedup vs baseline: 1.0210x; 1.0153x over previous
"""Bahdanau-style additive attention on 8 TRN2 NeuronCores (raw Bass).

Math (per batch b):
  e_proj[s,k] = sum_h e[s,h] * W[k,h]          (We = W[:, :512])
  d_proj[t,k] = sum_h d[t,h] * W[k,512+h]      (Wd = W[:, 512:])
  scores[s,t] = sum_k v[k] * tanh(e_proj[s,k] + d_proj[t,k] + b[k])
  attn        = log_softmax(scores, axis=s)
  out[t,h]    = sum_s attn[s,t] * e[s,h]

Sharding: 8 cores = 4 batches x 2 halves of tl (128 t per core).
Fully data-parallel, no collectives.

Device layout: k on partitions (4 chunks of 128).  Per t-tile of 8 t:
DVE tensor_scalar broadcast-adds build a [128, 8192] bf16 sum tile
(triple buffered), ScalarE tanh's it per half-tile (strided FD=4096
instructions), PE reduces against v (m=1 matmuls into [1,1024] PSUM
strips, triple buffered), DVE drains strips (bf16) into a rolling
[1, 8192] buffer scattered by 4 SBUF->SBUF DMAs into scores[t,s].
Epilogue: exp with accum_out (no max shift needed, |scores| <= ~8);
PE transposes f32 scores and computes the raw context matmul; the
log-softmax correction is applied on the HOST via linearity:
  ctx = scoresT @ e - ln(sumexp) (x) (sum_s e)
so the device ships raw ctx plus sumexp as out[:, 512] (also saves the
Ln activation-table switch and gains f64 accuracy).

Raw Bass with manual semaphores: this toolchain's walrus rejects any
instruction carrying more than one sync wait, so every wait is an
explicit single-semaphore wait_ge and engines are hand-pipelined
(software pipelining: DVE emits adds(tt,half) before the lagged strip
drains; ScalarE is the bottleneck engine at ~96% occupancy).
"""

import numpy as np
import ml_dtypes

import concourse.bass as bass
from concourse import mybir

F32 = mybir.dt.float32
BF16 = mybir.dt.bfloat16
AF = mybir.ActivationFunctionType

H = 512        # hidden
SL = 256       # source length (softmax dim)
TLC = 128      # target positions per core
P = 128        # partitions
KC = 4         # k chunks of 128
HCN = 4        # h chunks of 128
TT = 8         # t per tile
NTT = TLC // TT   # 16 t-tiles
JG = 4         # t per psum strip
BLK = TT * SL  # 2048
NSTR = TLC // JG  # 32 strips
SCQ = 8        # strips per scatter (32 scores rows)

# single bf16 mega-input tensor, loaded by TWO DMAs so the e-projection
# can start while the rest streams in: dma1 = [WE|ET] (cols 0:3072),
# dma2 = [WD|DT|V|f32 section] (cols 3072:).
O_WE, O_ET, O_WD, O_DT, O_V = 0, 2048, 3072, 5120, 5632
SPLIT = 3072
F0_E32, F0_B, F0_ID = 2818, 3842, 3846   # f32-unit offsets (byte 5636*2)
NBF = 7948


def build_nc():
    nc = bass.Bass("TRN2", target_bir_lowering=False, debug=False, num_devices=8)

    bf_d = nc.dram_tensor("bfh", [P, NBF], BF16, kind="ExternalInput").ap()
    out_d = nc.dram_tensor("out", [TLC, H + 1], F32, kind="ExternalOutput").ap()

    with (
        nc.sbuf_tensor("bf_sb", [P, NBF], BF16) as bf_sb,
        nc.sbuf_tensor("st0", [P, KC * BLK], BF16) as st0,
        nc.sbuf_tensor("st1", [P, KC * BLK], BF16) as st1,
        nc.sbuf_tensor("th0", [P, KC * BLK], BF16) as th0,
        nc.sbuf_tensor("th1", [P, KC * BLK], BF16) as th1,
        nc.sbuf_tensor("strips", [1, TLC * SL], BF16) as strips_sb,
        nc.sbuf_tensor("eprojT", [P, KC * SL], BF16) as eprojT_sb,
        nc.sbuf_tensor("biasd", [P, KC * TLC], F32) as biasd_sb,
        nc.sbuf_tensor("scores", [P, SL], BF16) as scores_sb,
        nc.sbuf_tensor("expt", [P, SL], F32) as expt_sb,
        nc.sbuf_tensor("sumexp", [P, 1], F32) as sumexp_sb,
        nc.sbuf_tensor("lse", [P, 1], F32) as lse_sb,
        nc.sbuf_tensor("attn", [P, SL], F32) as attn_sb,
        nc.sbuf_tensor("attnT", [P, 2 * P], F32) as attnT_sb,
        nc.sbuf_tensor("outsb", [P, H], F32) as out_sb,
        nc.psum_tensor("psA0", [P, 512], F32) as psA0,
        nc.psum_tensor("psA1", [P, 512], F32) as psA1,
        nc.psum_tensor("psS0", [1, JG * SL], F32) as psS0,
        nc.psum_tensor("psS1", [1, JG * SL], F32) as psS1,
        nc.semaphore("s_in") as s_in,
        nc.semaphore("s_pa") as s_pa,
        nc.semaphore("s_pac") as s_pac,
        nc.semaphore("s_add") as s_add,
        nc.semaphore("s_tanh") as s_tanh,
        nc.semaphore("s_strip") as s_strip,
        nc.semaphore("s_drain") as s_drain,
        nc.semaphore("s_scat") as s_scat,
        nc.semaphore("s_lse") as s_lse,
        nc.semaphore("s_attn") as s_attn,
        nc.semaphore("s_tr") as s_tr,
        nc.semaphore("s_trc") as s_trc,
        nc.semaphore("s_ctx") as s_ctx,
        nc.semaphore("s_out") as s_out,
        nc.semaphore("s_done") as s_done,
        nc.Block() as block,
    ):
        f32v = bf_sb[:, :].bitcast(F32)
        psA = [psA0, psA1]
        psS = [psS0, psS1, psS2]
        st = [st0, st1, st2]
        th = [th0, th1, th2]

        def we(hc, kc):
            o = O_WE + hc * H + kc * P
            return bf_sb[:, o:o + P]

        def wd(hc, kc):
            o = O_WD + hc * H + kc * P
            return bf_sb[:, o:o + P]

        def et(hc):
            o = O_ET + hc * SL
            return bf_sb[:, o:o + SL]

        def dt(hc):
            o = O_DT + hc * TLC
            return bf_sb[:, o:o + TLC]

        def vcol(kc):
            return bf_sb[:, O_V + kc:O_V + kc + 1]

        def e32(sc):
            return f32v[:, F0_E32 + sc * H:F0_E32 + (sc + 1) * H]

        def bcol(kc):
            return f32v[:, F0_B + kc:F0_B + kc + 1]

        id32 = f32v[:, F0_ID:F0_ID + P]

        @block.sync
        def _(sync):
            sync.dma_start(out=bf_sb[:, 0:SPLIT],
                           in_=bf_d[:, 0:SPLIT]).then_inc(s_in, 16)
            sync.dma_start(out=bf_sb[:, SPLIT:],
                           in_=bf_d[:, SPLIT:]).then_inc(s_in2, 16)
            for q in range(NSTR // SCQ):
                sync.wait_ge(s_drain, SCQ * (q + 1))
                r0 = q * SCQ * JG
                sync.dma_start(
                    out=scores_sb[r0:r0 + SCQ * JG, :],
                    in_=strips_sb[:, :].rearrange("p (t s) -> p t s", t=SCQ * JG),
                ).then_inc(s_scat, 16)
            sync.wait_ge(s_out, 1)
            sync.dma_start(out=out_d[:, :], in_=out_sb[:, :]).then_inc(s_done, 16)
            sync.wait_ge(s_done, 16)

        @block.tensor
        def _(tensor):
            tensor.wait_ge(s_in, 16)
            # phase A interleaved: (eproj kc, dproj kc) pairs
            for g in range(2 * KC):
                kc = g // 2
                n = SL if g % 2 == 0 else TLC
                wsel = we if g % 2 == 0 else wd
                rhs = et if g % 2 == 0 else dt
                if g == 1:
                    tensor.wait_ge(s_in2, 16)
                if g >= 2:
                    tensor.wait_ge(s_pac, g - 1)
                for hc in reversed(range(HCN)):
                    mm = tensor.matmul(
                        psA[g % 2][:, 0:n], lhsT=wsel(hc, kc), rhs=rhs(hc),
                        start=(hc == HCN - 1), stop=(hc == 0))
                mm.then_inc(s_pa, 1)
            # main loop: v-reduction strips
            for tt in range(NTT):
                for half in range(TT // JG):
                    i = 2 * tt + half
                    tensor.wait_ge(s_tanh, i + 1)
                    if i >= 3:
                        tensor.wait_ge(s_drain, i - 2)
                    for blk in range(JG * SL // 512):
                        col0 = half * JG * SL + blk * 512
                        for kc in reversed(range(KC)):
                            mm = tensor.matmul(
                                psS[i % 3][:, blk * 512:(blk + 1) * 512],
                                lhsT=vcol(kc),
                                rhs=th[tt % 3][:, kc * BLK + col0:kc * BLK + col0 + 512],
                                start=(kc == KC - 1), stop=(kc == 0))
                    mm.then_inc(s_strip, 1)
            # epilogue: transposes + fp32 context matmul (raw scores)
            tensor.wait_ge(s_pac, 2 * KC)
            tensor.wait_ge(s_sc32, 1)
            for sc in range(2):
                tensor.transpose(
                    psA[sc][:, 0:P], scores32_sb[:, sc * P:(sc + 1) * P], id32,
                ).then_inc(s_tr, 1)
            tensor.wait_ge(s_trc, 2)
            for sc in reversed(range(2)):
                mm = tensor.matmul(
                    psA0[:, 0:H], lhsT=attnT_sb[:, sc * P:(sc + 1) * P],
                    rhs=e32(sc), start=(sc == 1), stop=(sc == 0))
            mm.then_inc(s_ctx, 1)

        @block.vector
        def _(vector):
            vector.wait_ge(s_in2, 16)
            # phase A consumers
            for g in range(2 * KC):
                kc = g // 2
                vector.wait_ge(s_pa, g + 1)
                if g % 2 == 0:
                    ins = vector.tensor_copy(
                        eprojT_sb[:, kc * SL:(kc + 1) * SL], psA[g % 2][:, 0:SL])
                else:
                    ins = vector.tensor_scalar_add(
                        biasd_sb[:, kc * TLC:(kc + 1) * TLC],
                        psA[g % 2][:, 0:TLC], bcol(kc))
                ins.then_inc(s_pac, 1)
            # main loop: adds(tt,half) then drain of (tt-1,half) —
            # software pipelining at half-tile granularity
            def drain_one(i):
                vector.wait_ge(s_strip, i + 1)
                if i >= SCQ:
                    vector.wait_ge(s_scat, 16 * (i // SCQ))
                o = (i % SCQ) * JG * SL
                vector.tensor_copy(
                    strips_sb[:, o:o + JG * SL], psS[i % 3][:, :]
                ).then_inc(s_drain, 1)

            for tt in range(NTT):
                for half in range(2):
                    if tt >= 2:
                        vector.wait_ge(s_tanh, 2 * (tt - 2) + half + 1)
                    for kc in range(KC):
                        if tt == 0 and half == 0:
                            vector.wait_ge(s_pac, 2 * kc + 2)
                        for j in range(half * TT // 2, (half + 1) * TT // 2):
                            o = kc * BLK + j * SL
                            ts = vector.tensor_scalar_add(
                                st[tt % 3][:, o:o + SL],
                                eprojT_sb[:, kc * SL:(kc + 1) * SL],
                                biasd_sb[:, kc * TLC + tt * TT + j:kc * TLC + tt * TT + j + 1])
                    ts.then_inc(s_add, 1)
                    if tt >= 1:
                        drain_one(2 * (tt - 1) + half)
            drain_one(2 * NTT - 2)
            drain_one(2 * NTT - 1)
            # epilogue: f32 scores for the transposes, raw ctx + sumexp out
            vector.wait_ge(s_scat, 16 * (NSTR // SCQ))
            vector.tensor_copy(scores32_sb[:, :], scores_sb[:, :]).then_inc(s_sc32, 1)
            for sc in range(2):
                vector.wait_ge(s_tr, sc + 1)
                vector.tensor_copy(
                    attnT_sb[:, sc * P:(sc + 1) * P], psA[sc][:, 0:P],
                ).then_inc(s_trc, 1)
            vector.wait_ge(s_exp, 1)
            vector.tensor_copy(out_sb[:, H:H + 1], sumexp_sb[:, 0:1])
            vector.wait_ge(s_ctx, 1)
            vector.tensor_copy(out_sb[:, 0:H], psA0[:, 0:H]).then_inc(s_out, 1)

        @block.scalar
        def _(scalar):
            for tt in range(NTT):
                for half in range(2):
                    if tt >= 3:
                        scalar.wait_ge(s_strip, 2 * (tt - 3) + half + 1)
                    c0, c1 = half * JG * SL, (half + 1) * JG * SL
                    stv = st[tt % 3][:, :].rearrange("p (k c) -> p k c", k=KC)
                    thv = th[tt % 3][:, :].rearrange("p (k c) -> p k c", k=KC)
                    act = scalar.activation(
                        thv[:, :, c0:c1], stv[:, :, c0:c1], AF.Tanh)
                    act._wait_ge(s_add, 2 * tt + half + 1)
                    act.then_inc(s_tanh, 1)
            scalar.wait_ge(s_scat, 16 * (NSTR // SCQ))
            scalar.activation(expt_sb[:, :], scores_sb[:, :], AF.Exp,
                              accum_out=sumexp_sb[:, 0:1]).then_inc(s_exp, 1)

    return nc


_NC_CACHE = None


def _get_nc():
    global _NC_CACHE
    if _NC_CACHE is None:
        _NC_CACHE = build_nc()
    return _NC_CACHE


def _fold_chunks(a, n_chunks):
    """(n_chunks*128, F) -> (128, n_chunks*F) with chunk c at cols [c*F,(c+1)*F)."""
    ck = np.asarray(a).reshape(n_chunks, P, -1)
    return np.concatenate([ck[c] for c in range(n_chunks)], axis=1)


def make_in_maps(in_e, out_e, out_d, W, b, v):
    bf = ml_dtypes.bfloat16
    e = np.ascontiguousarray(out_e.transpose(1, 0, 2))  # (4, 256, 512) f32
    d = np.ascontiguousarray(out_d.transpose(1, 0, 2))  # (4, 256, 512) f32
    WeTh = _fold_chunks(W[:, :H].T, HCN).astype(bf)     # (128, 2048)
    WdTh = _fold_chunks(W[:, H:].T, HCN).astype(bf)
    bh = np.ascontiguousarray(b.reshape(KC, P).T).astype(np.float32)
    vh = np.ascontiguousarray(v.reshape(KC, P).T).astype(bf)
    ident = np.eye(P, dtype=np.float32)
    in_maps = []
    for c in range(8):
        bi, th_ = c // 2, c % 2
        eb = e[bi]                                  # (256, 512)
        db = d[bi, th_ * TLC:(th_ + 1) * TLC]       # (128, 512)
        f32_sec = np.concatenate(
            [_fold_chunks(eb, 2), bh, ident], axis=1).astype(np.float32)
        # round to bf16 precision so the bf16 view has no NaN patterns
        f32_sec = f32_sec.astype(bf).astype(np.float32)
        bf_all = np.concatenate(
            [WeTh, _fold_chunks(eb.T, HCN).astype(bf), WdTh,
             _fold_chunks(db.T, HCN).astype(bf), vh,
             f32_sec.view(bf)], axis=1)
        assert bf_all.shape[1] == NBF, bf_all.shape
        in_maps.append({"bfh": np.ascontiguousarray(bf_all)})
    return in_maps


def kernel(in_e, out_e, out_d, W, b, v):
    from concourse.bass_utils import run_bass_kernel_spmd
    nc = _get_nc()
    in_maps = make_in_maps(in_e, np.asarray(out_e, dtype=np.float32),
                           np.asarray(out_d, dtype=np.float32),
                           np.asarray(W, dtype=np.float32),
                           np.asarray(b, dtype=np.float32),
                           np.asarray(v, dtype=np.float32))
    res = run_bass_kernel_spmd(nc, in_maps, core_ids=list(range(8)))
    e = np.asarray(out_e, dtype=np.float64).transpose(1, 0, 2)  # (4, 256, 512)
    full = np.empty((SL, 4, H), dtype=np.float32)
    for c in range(8):
        bi, th_ = c // 2, c % 2
        o = res.results[c]["out"].astype(np.float64)
        raw, sumexp = o[:, :H], o[:, H]
        # log_softmax linearity: ctx = scoresT@e - ln(sumexp) x (sum_s e)
        E = e[bi].sum(axis=0)
        full[th_ * TLC:(th_ + 1) * TLC, bi, :] = (
            raw - np.log(sumexp)[:, None] * E[None, :]).astype(np.float32)
    return full
